# revision 36
# baseline (speedup 1.0000x reference)
"""Trainium2 Bass kernel for nn_Actor (data-parallel over 8 NeuronCores).

The reference network is entirely linear before its softmaxes (eval-mode BN is
affine; both heads act on a concat of affine projections), so it folds on the
host into per-channel coefficients:
    q_c = sum_k X_k * AQ[k, c] + cq[c]   (9 quant logits)
    z   = sum_k X_k * AZ[k]    + cz      (bandwidth logit difference)
with X = (s0, s1, s2, D0, D1, D2); D* is the scanned dyn state.

Device layout (per core, B_c = 1024, N = 50, T = 20):
  * G = 10 token groups, M = 5120 matmul columns per step; token (g, m):
    b = (m // 640) * 128 + (m % 128), n = ((m // 128) % 5) * 10 + g.
  * X-slab SBUF [61, M]: rows 0-29 static (k*10+g, streamed per step), row 30
    ones, rows 31-60 dyn (ch*10+g).
  * One fp32 matmul [61, M] x [61, 100] -> PSUM channel-major
    (out col j = g*10 + c; c = 0..8 quant logits, c = 9 -> z).
  * PE transposes [100, 128] -> token-major PSUM; softmax-9 / argmax / n-axis
    softmax / physics run token-major; PE transposes the next dyn state back.
"""
import numpy as np

LAST_EXEC_NS = None
LAST_TRACE = None
import concourse.bass as bass
from concourse import bacc
import concourse.mybir as mybir
from concourse.tile import TileContext
from concourse.bass_utils import run_bass_kernel_spmd
import concourse.bass_utils as _bu

_orig_run_command = _bu.run_command
def _patched_run_command(cmd, *a, **k):
    if isinstance(cmd, list):
        cmd = [x.replace("--enable-ldw-opt=false", "--enable-ldw-opt=true")
               if isinstance(x, str) else x for x in cmd]
    return _orig_run_command(cmd, *a, **k)
_bu.run_command = _patched_run_command

F32 = mybir.dt.float32
U32 = mybir.dt.uint32
AF = mybir.ActivationFunctionType
OP = mybir.AluOpType
AX = mybir.AxisListType

B, N, T = 8192, 50, 20
NCORES = 8
BC = B // NCORES          # 1024
G = 10
M = BC * N // G           # 5120
NBLK = M // 128           # 40
W = NBLK * G              # 400 tokens per partition per step
CH = 8                    # column chunks of 640 (one b-group each)
K_PTR = float(np.log(2.0) / 320.0)
NROWS, NOUT = 62, 100


def build_core():
    nc = bacc.Bacc()
    xs_d = nc.declare_dram_parameter("xs", [T, 30, M], F32, isOutput=False)
    s0t_d = nc.declare_dram_parameter("s0t", [T, 128, W], F32, isOutput=False)
    s2t_d = nc.declare_dram_parameter("s2t", [T, 128, W], F32, isOutput=False)

    cc_d = nc.declare_dram_parameter("cc", [128, 128 + 16 + 9 + NOUT + 1200 + M], F32, isOutput=False)
    out_d = nc.declare_dram_parameter("out", [T, 128, 3 * W], F32, isOutput=True)

    with (
        nc.sbuf_tensor([NROWS, M], F32) as xslab,
        nc.sbuf_tensor([NROWS, NOUT], F32) as wsb,
        nc.sbuf_tensor([128, 128], F32) as ident,
        nc.sbuf_tensor([128, 16], F32) as cst,
        nc.sbuf_tensor([128, 9], F32) as revidx,
        nc.sbuf_tensor([2, 128, W], F32) as s0t,
        nc.sbuf_tensor([2, 128, W], F32) as s2t,
        nc.sbuf_tensor([2, 128, NBLK * 30], F32) as dmat,
        nc.sbuf_tensor([100, M], F32) as qcm,
        nc.sbuf_tensor([128, W, 9], F32) as s9,
        nc.sbuf_tensor([128, W, 9], F32) as e9,
        nc.sbuf_tensor([128, W], F32) as msb,
        nc.sbuf_tensor([128, W], F32) as S9sb,
        nc.sbuf_tensor([128, W], F32) as rsb,
        nc.sbuf_tensor([128, W], F32) as isn,
        nc.sbuf_tensor([128, W], F32) as ensb,
        nc.sbuf_tensor([128, W], F32) as epsb,
        nc.sbuf_tensor([128, W], F32) as p1sb,
        nc.sbuf_tensor([128, W], F32) as t0sb,
        nc.sbuf_tensor([128, W], F32) as usb,
        nc.sbuf_tensor([128, W], F32) as ursb,
        nc.sbuf_tensor([128, W], F32) as xxsb,
        nc.sbuf_tensor([128, W], F32) as Lsb,
        nc.sbuf_tensor([128, W], F32) as wwsb,
        nc.sbuf_tensor([128, W], F32) as wrsb,
        nc.sbuf_tensor([128, W], F32) as vsb,
        nc.sbuf_tensor([128, W], U32) as msku,
        nc.sbuf_tensor([128, W], U32) as msku2,
        nc.sbuf_tensor([128, 8], F32) as snsb,
        nc.sbuf_tensor([128, 8], F32) as snrsb,
        nc.sbuf_tensor([128, 8], F32) as d0sb,
        nc.sbuf_tensor([128, 8], F32) as d0psb,
        nc.sbuf_tensor([128, 8], U32) as nrowu,
        nc.sbuf_tensor([2, 128, 3 * W], F32) as outsb,
        nc.psum_tensor([100, 640], F32) as pmm0,
        nc.psum_tensor([100, 640], F32) as pmm1,
        nc.psum_tensor([128, 500], F32) as ptm0,
        nc.psum_tensor([128, 500], F32) as ptm1,
        nc.psum_tensor([30, 640], F32) as pbk0,
        nc.psum_tensor([30, 640], F32) as pbk1,
        TileContext(nc) as tc,
    ):
        pmm, ptm, pbk = [pmm0, pmm1, pmm2], [ptm0, ptm1], [pbk0, pbk0]
        NANC, INFC, C17, B500, ONE = (cst[:, i:i + 1] for i in range(5))

        nc.sync.dma_start(out=ccsb[:], in_=cc_d[:])
        nc.vector.tensor_copy(dmat[:, 0], ccsb[:, 253:1453])
        nc.vector.tensor_copy(cstb[:], ccsb[:, 0:253])
        nc.vector.tensor_copy(xslab[:], ccsb[0:NROWS, 1453:1453 + M])
        nc.sync.dma_start(out=s0t[:, 0], in_=s0t_d[0])
        nc.sync.dma_start(out=s2t[:, 0], in_=s2t_d[0])

        for t in range(T):
            pp = t % 2
            dm_cur, dm_nxt = dmat[:, pp], dmat[:, 1 - pp]
            osb = outsb[:, pp]
            s0c, s2c = s0t[:, pp], s2t[:, pp]
            if t + 1 < T:
                nc.sync.dma_start(out=s0t[:, 1 - pp], in_=s0t_d[t + 1])
                nc.sync.dma_start(out=s2t[:, 1 - pp], in_=s2t_d[t + 1])

            # ---- matmul + fwd transpose + per-chunk channel ops ----
            # PE stream pipelined one chunk deep: mm(ci+1) issues before
            # transpose(ci) so PE never stalls on the psum->sbuf copy.
            def emit_mm(ci):
                pm = pmm[ci % 3]
                nc.tensor.matmul(pm[:], wsb, xslab[:, ci * 512:(ci + 1) * 512],
                                 start=True, stop=True)
                nc.vector.tensor_copy(qcm[:, ci * 512:(ci + 1) * 512], pm[:])

            def emit_chunk(ci):
                pt = ptm[ci % 2]
                for j in range(4):
                    nc.tensor.transpose(pt[:, j * 100:(j + 1) * 100],
                                        qcm[:, ci * 512 + j * 128:ci * 512 + (j + 1) * 128],
                                        ident[0:100, 0:100])
                qv = pt.ap().rearrange("p (nb g c) -> p nb g c", g=10, c=10)
                q9 = qv[:, :, :, 0:9].rearrange("p nb g c -> p (nb g) c")
                zv = qv[:, :, :, 9].rearrange("p nb g -> p (nb g)")
                wsl = slice(ci * 40, (ci + 1) * 40)
                mloc = msb[:, wsl]
                nc.vector.tensor_reduce(out=mloc, in_=q9, axis=AX.X, op=OP.max)
                s9v = s9[:, wsl, :]
                e9v = e9[:, wsl, :]
                nc.vector.tensor_tensor(out=s9v, in0=q9,
                                        in1=mloc[:, :, None].broadcast_to([128, 40, 9]),
                                        op=OP.subtract)
                nc.scalar.activation(out=e9v, in_=s9v, func=AF.Exp)
                nc.vector.tensor_reduce(out=S9sb[:, wsl], in_=e9v, axis=AX.X, op=OP.add)
                nc.vector.scalar_tensor_tensor(out=s9v, in0=s9v, scalar=0.0,
                                               in1=revidx[:, None, :].broadcast_to([128, 40, 9]),
                                               op0=OP.is_ge, op1=OP.mult)
                nc.vector.tensor_reduce(out=rsb[:, wsl], in_=s9v, axis=AX.X, op=OP.max)
                nc.scalar.activation(out=ensb[:, wsl], in_=zv, func=AF.Exp, scale=-1.0)

            emit_mm(0)
            emit_mm(1)
            for ci in range(2, 10):
                emit_mm(ci)
                emit_chunk(ci - 2)
            emit_chunk(8)
            emit_chunk(9)
            # ---- full-step token-major chain ----
            # (ACT ops ordered to cluster same-function calls: Exp... Ln... Exp)
            # p1 = 1 / (1 + exp(-z)); ep = exp(p1)
            nc.vector.tensor_scalar_add(out=t0sb[:], in0=ensb[:], scalar1=1.0)
            nc.vector.reciprocal(out=p1sb[:], in_=t0sb[:])
            nc.scalar.activation(out=epsb[:], in_=p1sb[:], func=AF.Exp)
            # physics u
            d1v = dm_cur.rearrange("p (nb c g) -> p nb c g", c=3, g=10)[:, :, 1]
            v3 = lambda ap: ap.rearrange("p (nb g) -> p nb g", g=10)
            nc.vector.scalar_tensor_tensor(out=v3(t0sb.ap()), in0=v3(s2c), scalar=0.05,
                                           in1=d1v, op0=OP.mult, op1=OP.add)   # d2pre
            nc.scalar.activation(out=usb[:], in_=t0sb[:], func=AF.Square, bias=B500)
            nc.vector.tensor_scalar_add(out=usb[:], in0=usb[:], scalar1=100.0)
            nc.vector.reciprocal(out=ursb[:], in_=usb[:])
            nc.vector.scalar_tensor_tensor(out=xxsb[:], in0=s0c, scalar=1e7, in1=ursb[:],
                                           op0=OP.mult, op1=OP.mult)
            nc.vector.tensor_scalar_add(out=t0sb[:], in0=xxsb[:], scalar1=1.0)
            # ptr = 17 - r - 9*isnan(S9)
            nc.vector.tensor_tensor(out=isn[:], in0=S9sb[:], in1=S9sb[:], op=OP.not_equal)
            nc.vector.scalar_tensor_tensor(out=rsb[:], in0=isn[:], scalar=9.0, in1=rsb[:],
                                           op0=OP.mult, op1=OP.add)
            nc.scalar.activation(out=osb[:, 0:W], in_=rsb[:], func=AF.Identity,
                                 bias=C17, scale=-1.0)
            # --- Ln cluster: log_q, ln(u) for d2n, ln(1+x) ---
            nc.scalar.activation(out=wwsb[:], in_=S9sb[:], func=AF.Ln)
            nc.vector.tensor_scalar_mul(out=osb[:, W:2 * W], in0=wwsb[:], scalar1=-1.0)
            nc.scalar.activation(out=Lsb[:], in_=usb[:], func=AF.Ln)  # ln(u) (d2n)
            nc.scalar.activation(out=ursb[:], in_=t0sb[:], func=AF.Ln)  # ln(1+x) (rate)
            nc.vector.tensor_scalar_add(out=t0sb[:], in0=t0sb[:], scalar1=-1.0)  # yd
            nc.vector.tensor_scalar_mul(out=xxsb[:], in0=t0sb[:], scalar1=0.9995)
            nc.vector.tensor_scalar(out=msku2[:], in0=t0sb[:], scalar1=1e-3, scalar2=None,
                                    op0=OP.is_lt)
            nc.vector.copy_predicated(out=ursb[:], mask=msku2[:], data=xxsb[:])
            # bdw = ep / sum_n ep
            epv = epsb.ap().rearrange("p (bb nb g) -> p bb (nb g)", bb=8, nb=5)
            nc.vector.tensor_reduce(out=snsb[:], in_=epv, axis=AX.X, op=OP.add)
            nc.vector.reciprocal(out=snrsb[:], in_=snsb[:])
            nc.vector.tensor_tensor(out=osb[:, 2 * W:3 * W].rearrange("p (a b) -> p a b", a=8),
                                    in0=epsb.ap().rearrange("p (a b) -> p a b", a=8),
                                    in1=snrsb[:, :, None].broadcast_to([128, 8, 50]),
                                    op=OP.mult)
            # d2n = exp(0.5 ln u) -> dm_nxt ch2
            d2slot = dm_nxt.rearrange("p (nb c g) -> p nb c g", c=3, g=10)[:, :, 2]
            nc.scalar.activation(out=d2slot, in_=v3(Lsb.ap()), func=AF.Exp, scale=0.5)
            # w = bdw * L ; wr = 1/w (exact: 0 -> inf, NaN -> NaN)
            nc.vector.tensor_tensor(out=wwsb[:], in0=osb[:, 2 * W:3 * W], in1=ursb[:], op=OP.mult)
            nc.vector.reciprocal(out=wrsb[:], in_=wwsb[:])
            # v = ptr * (ln2/320) * wr
            nc.vector.scalar_tensor_tensor(out=vsb[:], in0=osb[:, 0:W], scalar=K_PTR, in1=wrsb[:],
                                           op0=OP.mult, op1=OP.mult)
            # d0 = max_n v (+ NaN repair since DVE max drops NaN)
            vv = vsb.ap().rearrange("p (bb nb g) -> p bb (nb g)", bb=8, nb=5)
            nc.vector.tensor_reduce(out=d0sb[:], in_=vv, axis=AX.X, op=OP.max)
            nc.vector.tensor_tensor(out=msku[:], in0=vsb[:], in1=vsb[:], op=OP.not_equal)
            nc.vector.tensor_reduce(out=nrowu[:],
                                    in_=msku.ap().rearrange("p (bb nb g) -> p bb (nb g)", bb=8, nb=5),
                                    axis=AX.X, op=OP.max)
            nc.vector.copy_predicated(out=d0sb[:], mask=nrowu[:],
                                      data=NANC.broadcast_to([128, 8]))
            nc.vector.tensor_scalar_add(out=d0psb[:], in0=d0sb[:], scalar1=0.05)
            if t + 1 < T:
                # D0 slots (token-major) + D1 = d0p * s2 + D1_cur
                d0bc5 = d0psb[:, :, None, None].broadcast_to([128, 8, 5, 10])
                d0slot = dm_nxt.rearrange("p (bb nb c g) -> p bb nb c g", bb=8, c=3, g=10)[:, :, :, 0]
                nc.vector.tensor_copy(d0slot, d0bc5)
                d1slot = dm_nxt.rearrange("p (bb nb c g) -> p bb nb c g", bb=8, c=3, g=10)[:, :, :, 1]
                v5 = lambda ap: ap.rearrange("p (bb nb g) -> p bb nb g", bb=8, g=10)
                nc.vector.tensor_tensor(out=v5(t0sb.ap()), in0=v5(s2c), in1=d0bc5, op=OP.mult)
                nc.vector.tensor_tensor(out=d1slot, in0=v5(t0sb.ap()),
                                        in1=dm_cur.rearrange("p (bb nb c g) -> p bb nb c g", bb=8, c=3, g=10)[:, :, :, 1],
                                        op=OP.add)
                # back-transpose dyn state to channel-major X rows
                for ci in range(CH):
                    pb = pbk[ci % 2]
                    for j in range(5):
                        blk = ci * 5 + j
                        nc.tensor.transpose(pb[:, j * 128:(j + 1) * 128],
                                            dm_nxt[:, blk * 30:(blk + 1) * 30],
                                            ident[:, 0:128])
                    nc.scalar.copy(out=xslab[32:62, ci * 640:(ci + 1) * 640],
                                   in_=pb[:, 0:640])
                # stream next static rows (after this step's matmuls consumed them)
                nc.sync.dma_start(out=xslab[0:30, :], in_=xs_d[t + 1])
            nc.sync.dma_start(out=out_d[t], in_=osb)
    return nc


# ------------------------- host side -------------------------

def _fold_weights(Ws, bs, Wd, bd, Wq, bq, Wb, bb, g_s, beta_s, m_s, v_s,
                  g_d, beta_d, m_d, v_d):
    f32 = np.float32
    EPS = f32(1e-5)
    H = Ws.shape[0]
    sc_s = g_s / np.sqrt(v_s + EPS)
    off_s = beta_s - m_s * sc_s
    sc_d = g_d / np.sqrt(v_d + EPS)
    off_d = beta_d - m_d * sc_d
    Aqs = ((sc_s[:, None] * Ws.T) @ Wq[:, :H].T).astype(f32)
    Aqd = ((sc_d[:, None] * Wd.T) @ Wq[:, H:].T).astype(f32)
    cq = ((off_s @ Ws.T + bs) @ Wq[:, :H].T + (off_d @ Wd.T + bd) @ Wq[:, H:].T + bq).astype(f32)
    Abs_ = ((sc_s[:, None] * Ws.T) @ Wb[:, :H].T).astype(f32)
    Abd = ((sc_d[:, None] * Wd.T) @ Wb[:, H:].T).astype(f32)
    cb = ((off_s @ Ws.T + bs) @ Wb[:, :H].T + (off_d @ Wd.T + bd) @ Wb[:, H:].T + bb).astype(f32)
    As_z = (Abs_[:, 1] - Abs_[:, 0]).astype(f32)
    Ad_z = (Abd[:, 1] - Abd[:, 0]).astype(f32)
    cz = f32(cb[1] - cb[0])
    return Aqs, Aqd, cq, As_z, Ad_z, cz


def _stationary(Aqs, Aqd, cq, As_z, Ad_z, cz):
    Wst = np.zeros((NROWS, NOUT), np.float32)
    for g in range(G):
        for c in range(9):
            j = g * 10 + c
            for k in range(3):
                Wst[k * 10 + g, j] = Aqs[k, c]
                Wst[32 + k * 10 + g, j] = Aqd[k, c]
            Wst[30, j] = cq[c]
        j = g * 10 + 9
        for k in range(3):
            Wst[k * 10 + g, j] = As_z[k]
            Wst[32 + k * 10 + g, j] = Ad_z[k]
        Wst[30, j] = cz
    return Wst


# token mapping per core: m, g <-> (b, n)
_mm = np.arange(M)
_TOK_B = (_mm // 640) * 128 + (_mm % 128)          # [M]
_TOK_NB = (_mm // 128) % 5                          # [M]
# token-major index: partition p = b % 128, w = (b//128)*50 + nb*10 + g


def _prep_core(static_c, dyn0_c, Wst, consts):
    """Build the per-core in_map. static_c [BC,N,3,T], dyn0_c [BC,N,3]."""
    f32 = np.float32
    xs = np.empty((T, 30, M), f32)
    # row k*10+g, col m = static[b(m), nb(m)*10+g, k, t]
    for g in range(G):
        n_idx = _TOK_NB * 10 + g
        sl = static_c[_TOK_B, n_idx, :, :]          # [M, 3, T]
        for k in range(3):
            xs[:, k * 10 + g, :] = sl[:, k, :].T
    # token-major static planes s0, s2: [T, 128, W]; w = (b//128)*50 + nb*10 + g
    bb = np.arange(BC) // 128
    pp = np.arange(BC) % 128
    s0t = np.empty((T, 128, W), f32)
    s2t = np.empty((T, 128, W), f32)
    wix = (bb[:, None] * 50 + (np.arange(N) // 10 * 10 + np.arange(N) % 10)[None, :])
    # n-index mapping: w = bb*50 + nb*10 + g with n = nb*10+g -> w = bb*50 + n
    for tt in range(T):
        s0t[tt, pp[:, None], bb[:, None] * 50 + np.arange(N)[None, :]] = static_c[:, :, 0, tt]
        s2t[tt, pp[:, None], bb[:, None] * 50 + np.arange(N)[None, :]] = static_c[:, :, 2, tt]
    # dyn0: X rows 30..60 [31, M]; row 30 ones; row 31+ch*10+g
    dyn0 = np.zeros((32, M), f32)
    dyn0[0, :] = 1.0
    for g in range(G):
        n_idx = _TOK_NB * 10 + g
        dl = dyn0_c[_TOK_B, n_idx, :]               # [M, 3]
        for ch in range(3):
            dyn0[2 + ch * 10 + g, :] = dl[:, ch]
    # dmat0 token-major [128, NBLK*30]: blk = bb*5+nb, slot c*10+g
    dmat0 = np.empty((128, NBLK, 3, G), f32)
    n_all = np.arange(N)
    blk = bb[:, None] * 5 + (n_all // 10)[None, :]  # [BC, N]
    gg = (n_all % 10)[None, :].repeat(BC, 0)
    for ch in range(3):
        dmat0[pp[:, None], blk, ch, gg] = dyn0_c[:, :, ch]
    dmat0 = dmat0.reshape(128, NBLK * 30)
    cc = np.zeros((128, 128 + 16 + 9 + NOUT + 1200 + M), f32)
    cc[:, 0:128] = consts["ident"]
    cc[:, 128:144] = consts["cst"]
    cc[:, 144:153] = consts["rvx"]
    cc[0:NROWS, 153:153 + NOUT] = Wst
    cc[:, 253:1453] = dmat0
    cc[0:30, 1453:1453 + M] = xs[0]
    cc[30:62, 1453:1453 + M] = dyn0
    return {"xs": xs, "s0t": s0t, "s2t": s2t, "cc": cc}


def kernel(static, dynamic, Ws, bs, Wd, bd, Wq, bq, Wb, bb, g_s, beta_s,
           m_s, v_s, g_d, beta_d, m_d, v_d):
    static = np.asarray(static, np.float32)
    dyn0 = np.asarray(dynamic, np.float32)[:, :, :, 0]
    Aqs, Aqd, cq, As_z, Ad_z, cz = _fold_weights(
        np.asarray(Ws), np.asarray(bs), np.asarray(Wd), np.asarray(bd),
        np.asarray(Wq), np.asarray(bq), np.asarray(Wb), np.asarray(bb),
        np.asarray(g_s), np.asarray(beta_s), np.asarray(m_s), np.asarray(v_s),
        np.asarray(g_d), np.asarray(beta_d), np.asarray(m_d), np.asarray(v_d))
    Wst = _stationary(Aqs, Aqd, cq, As_z, Ad_z, cz)
    cst = np.zeros((128, 16), np.float32)
    cst[:, 0] = np.nan
    cst[:, 1] = np.inf
    cst[:, 2] = 17.0
    cst[:, 3] = -500.0
    cst[:, 4] = 1.0
    rvx = np.tile((9.0 - np.arange(9, dtype=np.float32))[None, :], (128, 1))
    ident = np.eye(128, dtype=np.float32)
    consts = {"cst": cst, "rvx": rvx, "ident": ident}

    nc = build_core()
    if not nc.is_finalized():
        nc.finalize()
    in_maps = []
    for c in range(NCORES):
        sl = slice(c * BC, (c + 1) * BC)
        in_maps.append(_prep_core(static[sl], dyn0[sl], Wst, consts))
    import os
    trace = bool(os.environ.get("KERNEL_TRACE"))
    res = run_bass_kernel_spmd(nc, in_maps, list(range(NCORES)), trace=trace)
    if trace:
        global LAST_EXEC_NS, LAST_TRACE
        LAST_EXEC_NS = res.exec_time_ns
        LAST_TRACE = getattr(res, "instructions_and_trace", None)
        try:
            print("HW exec time:", res.exec_time_ns, "ns")
        except Exception:
            pass

    # ---- unscramble ----
    action = np.empty((B, N, 3, T), np.float32)
    logp = np.empty((B, N, 3, T), np.float32)
    bb_ = np.arange(BC) // 128
    pp_ = np.arange(BC) % 128
    wix = bb_[:, None] * 50 + np.arange(N)[None, :]   # [BC, N]
    for c in range(NCORES):
        o = res.results[c]["out"].reshape(T, 128, 3, W)
        sl = slice(c * BC, (c + 1) * BC)
        ptr = o[:, pp_[:, None], 0, wix]               # [T, BC, N]
        lq = o[:, pp_[:, None], 1, wix]
        bd_ = o[:, pp_[:, None], 2, wix]
        action[sl, :, 0, :] = 0.0
        action[sl, :, 1, :] = np.moveaxis(ptr, 0, -1)
        action[sl, :, 2, :] = np.moveaxis(bd_, 0, -1)
        logp[sl, :, 0, :] = np.float32(np.log(np.float32(0.5)))
        logp[sl, :, 1, :] = np.moveaxis(lq, 0, -1)
        logp[sl, :, 2, :] = np.moveaxis(bd_, 0, -1)
    return action, logp


# revision 38
# speedup vs baseline: 1.0570x; 1.0570x over previous
"""Trainium2 Bass kernel for nn_Actor (data-parallel over 8 NeuronCores).

The reference network is entirely linear before its softmaxes (eval-mode BN is
affine; both heads act on a concat of affine projections), so it folds on the
host into per-channel coefficients:
    q_c = sum_k X_k * AQ[k, c] + cq[c]   (9 quant logits)
    z   = sum_k X_k * AZ[k]    + cz      (bandwidth logit difference)
with X = (s0, s1, s2, D0, D1, D2); D* is the scanned dyn state.

Device layout (per core, B_c = 1024, N = 50, T = 20):
  * G = 10 token groups, M = 5120 matmul columns per step; token (g, m):
    b = (m // 640) * 128 + (m % 128), n = ((m // 128) % 5) * 10 + g.
  * X-slab SBUF [61, M]: rows 0-29 static (k*10+g, streamed per step), row 30
    ones, rows 31-60 dyn (ch*10+g).
  * One fp32 matmul [61, M] x [61, 100] -> PSUM channel-major
    (out col j = g*10 + c; c = 0..8 quant logits, c = 9 -> z).
  * PE transposes [100, 128] -> token-major PSUM; softmax-9 / argmax / n-axis
    softmax / physics run token-major; PE transposes the next dyn state back.
"""
import numpy as np

LAST_EXEC_NS = None
LAST_TRACE = None
import concourse.bass as bass
from concourse import bacc
import concourse.mybir as mybir
from concourse.tile import TileContext
from concourse.bass_utils import run_bass_kernel_spmd
import concourse.bass_utils as _bu

_orig_run_command = _bu.run_command
def _patched_run_command(cmd, *a, **k):
    if isinstance(cmd, list):
        cmd = [x.replace("--enable-ldw-opt=false", "--enable-ldw-opt=true")
               if isinstance(x, str) else x for x in cmd]
    return _orig_run_command(cmd, *a, **k)
_bu.run_command = _patched_run_command

F32 = mybir.dt.float32
U32 = mybir.dt.uint32
AF = mybir.ActivationFunctionType
OP = mybir.AluOpType
AX = mybir.AxisListType

B, N, T = 8192, 50, 20
NCORES = 8
BC = B // NCORES          # 1024
G = 10
M = BC * N // G           # 5120
NBLK = M // 128           # 40
W = NBLK * G              # 400 tokens per partition per step
CH = 8                    # column chunks of 640 (one b-group each)
K_PTR = float(np.log(2.0) / 320.0)
NROWS, NOUT = 62, 100


def build_core():
    nc = bacc.Bacc()
    xs_d = nc.declare_dram_parameter("xs", [T, 30, M], F32, isOutput=False)
    s0t_d = nc.declare_dram_parameter("s0t", [T, 128, W], F32, isOutput=False)
    s2t_d = nc.declare_dram_parameter("s2t", [T, 128, W], F32, isOutput=False)

    cc_d = nc.declare_dram_parameter("cc", [128, 128 + 16 + 9 + NOUT + 1200 + M], F32, isOutput=False)
    out_d = nc.declare_dram_parameter("out", [T, 128, 3 * W], F32, isOutput=True)

    with (
        nc.sbuf_tensor([NROWS, M], F32) as xslab,
        nc.sbuf_tensor([NROWS, NOUT], F32) as wsb,
        nc.sbuf_tensor([128, 128], F32) as ident,
        nc.sbuf_tensor([128, 16], F32) as cst,
        nc.sbuf_tensor([128, 9], F32) as revidx,
        nc.sbuf_tensor([2, 128, W], F32) as s0t,
        nc.sbuf_tensor([2, 128, W], F32) as s2t,
        nc.sbuf_tensor([2, 128, NBLK * 30], F32) as dmat,
        nc.sbuf_tensor([100, M], F32) as qcm,
        nc.sbuf_tensor([128, W, 9], F32) as s9,
        nc.sbuf_tensor([128, W, 9], F32) as e9,
        nc.sbuf_tensor([128, W], F32) as msb,
        nc.sbuf_tensor([128, W], F32) as S9sb,
        nc.sbuf_tensor([128, W], F32) as rsb,
        nc.sbuf_tensor([128, W], F32) as isn,
        nc.sbuf_tensor([128, W], F32) as ensb,
        nc.sbuf_tensor([128, W], F32) as epsb,
        nc.sbuf_tensor([128, W], F32) as p1sb,
        nc.sbuf_tensor([128, W], F32) as t0sb,
        nc.sbuf_tensor([128, W], F32) as usb,
        nc.sbuf_tensor([128, W], F32) as ursb,
        nc.sbuf_tensor([128, W], F32) as xxsb,
        nc.sbuf_tensor([128, W], F32) as Lsb,
        nc.sbuf_tensor([128, W], F32) as wwsb,
        nc.sbuf_tensor([128, W], F32) as wrsb,
        nc.sbuf_tensor([128, W], F32) as vsb,
        nc.sbuf_tensor([128, W], U32) as msku,
        nc.sbuf_tensor([128, W], U32) as msku2,
        nc.sbuf_tensor([128, 8], F32) as snsb,
        nc.sbuf_tensor([128, 8], F32) as snrsb,
        nc.sbuf_tensor([128, 8], F32) as d0sb,
        nc.sbuf_tensor([128, 8], F32) as d0psb,
        nc.sbuf_tensor([128, 8], U32) as nrowu,
        nc.sbuf_tensor([2, 128, 3 * W], F32) as outsb,
        nc.psum_tensor([100, 640], F32) as pmm0,
        nc.psum_tensor([100, 640], F32) as pmm1,
        nc.psum_tensor([128, 500], F32) as ptm0,
        nc.psum_tensor([128, 500], F32) as ptm1,
        nc.psum_tensor([30, 640], F32) as pbk0,
        nc.psum_tensor([30, 640], F32) as pbk1,
        TileContext(nc) as tc,
    ):
        pmm, ptm, pbk = [pmm0, pmm1], [ptm0, ptm1], [pbk0, pbk1]
        NANC, INFC, C17, B500, ONE = (cst[:, i:i + 1] for i in range(5))

        nc.sync.dma_start(out=ccsb[:], in_=cc_d[:])
        nc.vector.tensor_copy(dmat[:, 0], ccsb[:, 253:1453])
        nc.vector.tensor_copy(cstb[:], ccsb[:, 0:253])
        nc.vector.tensor_copy(xslab[:], ccsb[0:NROWS, 1453:1453 + M])
        nc.sync.dma_start(out=s0t[:, 0], in_=s0t_d[0])
        nc.sync.dma_start(out=s2t[:, 0], in_=s2t_d[0])

        for t in range(T):
            pp = t % 2
            dm_cur, dm_nxt = dmat[:, pp], dmat[:, 1 - pp]
            osb = outsb[:, pp]
            s0c, s2c = s0t[:, pp], s2t[:, pp]
            if t + 1 < T:
                nc.sync.dma_start(out=s0t[:, 1 - pp], in_=s0t_d[t + 1])
                nc.sync.dma_start(out=s2t[:, 1 - pp], in_=s2t_d[t + 1])

            # ---- matmul + fwd transpose + per-chunk channel ops ----
            # PE stream pipelined one chunk deep: mm(ci+1) issues before
            # transpose(ci) so PE never stalls on the psum->sbuf copy.
            def emit_mm(ci):
                pm = pmm[ci % 2]
                for a0, a1 in [(0, 512), (512, 640)]:
                    nc.tensor.matmul(pm[:, a0:a1], wsb,
                                     xslab[:, ci * 640 + a0:ci * 640 + a1],
                                     start=True, stop=True)
                nc.vector.tensor_copy(qcm[:, ci * 640:ci * 640 + 512], pm[:, 0:512])
                nc.scalar.copy(out=qcm[:, ci * 640 + 512:(ci + 1) * 640], in_=pm[:, 512:640])

            def emit_chunk(ci):
                pt = ptm[ci % 2]
                for j in range(5):
                    nc.tensor.transpose(pt[:, j * 100:(j + 1) * 100],
                                        qcm[:, ci * 640 + j * 128:ci * 640 + (j + 1) * 128],
                                        ident[0:100, 0:100])
                qv = pt.ap().rearrange("p (nb g c) -> p nb g c", g=10, c=10)
                q9 = qv[:, :, :, 0:9].rearrange("p nb g c -> p (nb g) c")
                zv = qv[:, :, :, 9].rearrange("p nb g -> p (nb g)")
                wsl = slice(ci * 50, (ci + 1) * 50)
                mloc = msb[:, wsl]
                nc.vector.tensor_reduce(out=mloc, in_=q9, axis=AX.X, op=OP.max)
                s9v = s9[:, wsl, :]
                e9v = e9[:, wsl, :]
                nc.vector.tensor_tensor(out=s9v, in0=q9,
                                        in1=mloc[:, :, None].broadcast_to([128, 50, 9]),
                                        op=OP.subtract)
                nc.scalar.activation(out=e9v, in_=s9v, func=AF.Exp)
                nc.vector.tensor_reduce(out=S9sb[:, wsl], in_=e9v, axis=AX.X, op=OP.add)
                nc.vector.scalar_tensor_tensor(out=s9v, in0=s9v, scalar=0.0,
                                               in1=revidx[:, None, :].broadcast_to([128, 50, 9]),
                                               op0=OP.is_ge, op1=OP.mult)
                nc.vector.tensor_reduce(out=rsb[:, wsl], in_=s9v, axis=AX.X, op=OP.max)
                nc.scalar.activation(out=ensb[:, wsl], in_=zv, func=AF.Exp, scale=-1.0)

            emit_mm(0)
            for ci in range(1, CH):
                emit_mm(ci)
                emit_chunk(ci - 1)
            emit_chunk(CH - 1)
            # ---- full-step token-major chain ----
            # (ACT ops ordered to cluster same-function calls: Exp... Ln... Exp)
            # p1 = 1 / (1 + exp(-z)); ep = exp(p1)
            nc.vector.tensor_scalar_add(out=t0sb[:], in0=ensb[:], scalar1=1.0)
            nc.vector.reciprocal(out=p1sb[:], in_=t0sb[:])
            nc.scalar.activation(out=epsb[:], in_=p1sb[:], func=AF.Exp)
            # physics u
            d1v = dm_cur.rearrange("p (nb c g) -> p nb c g", c=3, g=10)[:, :, 1]
            v3 = lambda ap: ap.rearrange("p (nb g) -> p nb g", g=10)
            nc.vector.scalar_tensor_tensor(out=v3(t0sb.ap()), in0=v3(s2c), scalar=0.05,
                                           in1=d1v, op0=OP.mult, op1=OP.add)   # d2pre
            nc.scalar.activation(out=usb[:], in_=t0sb[:], func=AF.Square, bias=B500)
            nc.vector.tensor_scalar_add(out=usb[:], in0=usb[:], scalar1=100.0)
            nc.vector.reciprocal(out=ursb[:], in_=usb[:])
            nc.vector.scalar_tensor_tensor(out=xxsb[:], in0=s0c, scalar=1e7, in1=ursb[:],
                                           op0=OP.mult, op1=OP.mult)
            nc.vector.tensor_scalar_add(out=t0sb[:], in0=xxsb[:], scalar1=1.0)
            # ptr = 17 - r - 9*isnan(S9)
            nc.vector.tensor_tensor(out=isn[:], in0=S9sb[:], in1=S9sb[:], op=OP.not_equal)
            nc.vector.scalar_tensor_tensor(out=rsb[:], in0=isn[:], scalar=9.0, in1=rsb[:],
                                           op0=OP.mult, op1=OP.add)
            nc.scalar.activation(out=osb[:, 0:W], in_=rsb[:], func=AF.Identity,
                                 bias=C17, scale=-1.0)
            # --- Ln cluster: log_q, ln(u) for d2n, ln(1+x) ---
            nc.scalar.activation(out=wwsb[:], in_=S9sb[:], func=AF.Ln)
            nc.vector.tensor_scalar_mul(out=osb[:, W:2 * W], in0=wwsb[:], scalar1=-1.0)
            nc.scalar.activation(out=Lsb[:], in_=usb[:], func=AF.Ln)  # ln(u) (d2n)
            nc.scalar.activation(out=ursb[:], in_=t0sb[:], func=AF.Ln)  # ln(1+x) (rate)
            nc.vector.tensor_scalar_add(out=t0sb[:], in0=t0sb[:], scalar1=-1.0)  # yd
            nc.vector.tensor_scalar_mul(out=xxsb[:], in0=t0sb[:], scalar1=0.9995)
            nc.vector.tensor_scalar(out=msku2[:], in0=t0sb[:], scalar1=1e-3, scalar2=None,
                                    op0=OP.is_lt)
            nc.vector.copy_predicated(out=ursb[:], mask=msku2[:], data=xxsb[:])
            # bdw = ep / sum_n ep
            epv = epsb.ap().rearrange("p (bb nb g) -> p bb (nb g)", bb=8, nb=5)
            nc.vector.tensor_reduce(out=snsb[:], in_=epv, axis=AX.X, op=OP.add)
            nc.vector.reciprocal(out=snrsb[:], in_=snsb[:])
            nc.vector.tensor_tensor(out=osb[:, 2 * W:3 * W].rearrange("p (a b) -> p a b", a=8),
                                    in0=epsb.ap().rearrange("p (a b) -> p a b", a=8),
                                    in1=snrsb[:, :, None].broadcast_to([128, 8, 50]),
                                    op=OP.mult)
            # d2n = exp(0.5 ln u) -> dm_nxt ch2
            d2slot = dm_nxt.rearrange("p (nb c g) -> p nb c g", c=3, g=10)[:, :, 2]
            nc.scalar.activation(out=d2slot, in_=v3(Lsb.ap()), func=AF.Exp, scale=0.5)
            # w = bdw * L ; wr = 1/w (exact: 0 -> inf, NaN -> NaN)
            nc.vector.tensor_tensor(out=wwsb[:], in0=osb[:, 2 * W:3 * W], in1=ursb[:], op=OP.mult)
            nc.vector.reciprocal(out=wrsb[:], in_=wwsb[:])
            # v = ptr * (ln2/320) * wr
            nc.vector.scalar_tensor_tensor(out=vsb[:], in0=osb[:, 0:W], scalar=K_PTR, in1=wrsb[:],
                                           op0=OP.mult, op1=OP.mult)
            # d0 = max_n v (+ NaN repair since DVE max drops NaN)
            vv = vsb.ap().rearrange("p (bb nb g) -> p bb (nb g)", bb=8, nb=5)
            nc.vector.tensor_reduce(out=d0sb[:], in_=vv, axis=AX.X, op=OP.max)
            nc.vector.tensor_tensor(out=msku[:], in0=vsb[:], in1=vsb[:], op=OP.not_equal)
            nc.vector.tensor_reduce(out=nrowu[:],
                                    in_=msku.ap().rearrange("p (bb nb g) -> p bb (nb g)", bb=8, nb=5),
                                    axis=AX.X, op=OP.max)
            nc.vector.copy_predicated(out=d0sb[:], mask=nrowu[:],
                                      data=NANC.broadcast_to([128, 8]))
            nc.vector.tensor_scalar_add(out=d0psb[:], in0=d0sb[:], scalar1=0.05)
            if t + 1 < T:
                # D0 slots (token-major) + D1 = d0p * s2 + D1_cur
                d0bc5 = d0psb[:, :, None, None].broadcast_to([128, 8, 5, 10])
                d0slot = dm_nxt.rearrange("p (bb nb c g) -> p bb nb c g", bb=8, c=3, g=10)[:, :, :, 0]
                nc.vector.tensor_copy(d0slot, d0bc5)
                d1slot = dm_nxt.rearrange("p (bb nb c g) -> p bb nb c g", bb=8, c=3, g=10)[:, :, :, 1]
                v5 = lambda ap: ap.rearrange("p (bb nb g) -> p bb nb g", bb=8, g=10)
                nc.vector.tensor_tensor(out=v5(t0sb.ap()), in0=v5(s2c), in1=d0bc5, op=OP.mult)
                nc.vector.tensor_tensor(out=d1slot, in0=v5(t0sb.ap()),
                                        in1=dm_cur.rearrange("p (bb nb c g) -> p bb nb c g", bb=8, c=3, g=10)[:, :, :, 1],
                                        op=OP.add)
                # back-transpose dyn state to channel-major X rows
                for ci in range(CH):
                    # alternate psum banks per transpose so group ci+1's
                    # transposes overlap group ci's copies
                    pa, pb = pbk[ci % 2], pbk[1 - ci % 2]
                    for j in range(5):
                        blk = ci * 5 + j
                        dst = pa if j % 2 == 0 else pb
                        nc.tensor.transpose(dst[:, (j // 2) * 128:(j // 2 + 1) * 128],
                                            dm_nxt[:, blk * 30:(blk + 1) * 30],
                                            ident[:, 0:128])
                    # even j (0,2,4) -> xslab cols {0,256,512}+128; odd j (1,3) -> {128,384}+128
                    xv = xslab[32:62, ci * 640:(ci + 1) * 640].rearrange("p (a b) -> p a b", b=128)
                    nc.scalar.copy(out=xv[:, 0::2, :], in_=pa[:, 0:384].rearrange("p (a b) -> p a b", b=128))
                    nc.scalar.copy(out=xv[:, 1::2, :], in_=pb[:, 0:256].rearrange("p (a b) -> p a b", b=128))
                # stream next static rows (after this step's matmuls consumed them)
                nc.sync.dma_start(out=xslab[0:30, :], in_=xs_d[t + 1])
            nc.sync.dma_start(out=out_d[t], in_=osb)
    return nc


# ------------------------- host side -------------------------

def _fold_weights(Ws, bs, Wd, bd, Wq, bq, Wb, bb, g_s, beta_s, m_s, v_s,
                  g_d, beta_d, m_d, v_d):
    f32 = np.float32
    EPS = f32(1e-5)
    H = Ws.shape[0]
    sc_s = g_s / np.sqrt(v_s + EPS)
    off_s = beta_s - m_s * sc_s
    sc_d = g_d / np.sqrt(v_d + EPS)
    off_d = beta_d - m_d * sc_d
    Aqs = ((sc_s[:, None] * Ws.T) @ Wq[:, :H].T).astype(f32)
    Aqd = ((sc_d[:, None] * Wd.T) @ Wq[:, H:].T).astype(f32)
    cq = ((off_s @ Ws.T + bs) @ Wq[:, :H].T + (off_d @ Wd.T + bd) @ Wq[:, H:].T + bq).astype(f32)
    Abs_ = ((sc_s[:, None] * Ws.T) @ Wb[:, :H].T).astype(f32)
    Abd = ((sc_d[:, None] * Wd.T) @ Wb[:, H:].T).astype(f32)
    cb = ((off_s @ Ws.T + bs) @ Wb[:, :H].T + (off_d @ Wd.T + bd) @ Wb[:, H:].T + bb).astype(f32)
    As_z = (Abs_[:, 1] - Abs_[:, 0]).astype(f32)
    Ad_z = (Abd[:, 1] - Abd[:, 0]).astype(f32)
    cz = f32(cb[1] - cb[0])
    return Aqs, Aqd, cq, As_z, Ad_z, cz


def _stationary(Aqs, Aqd, cq, As_z, Ad_z, cz):
    Wst = np.zeros((NROWS, NOUT), np.float32)
    for g in range(G):
        for c in range(9):
            j = g * 10 + c
            for k in range(3):
                Wst[k * 10 + g, j] = Aqs[k, c]
                Wst[32 + k * 10 + g, j] = Aqd[k, c]
            Wst[30, j] = cq[c]
        j = g * 10 + 9
        for k in range(3):
            Wst[k * 10 + g, j] = As_z[k]
            Wst[32 + k * 10 + g, j] = Ad_z[k]
        Wst[30, j] = cz
    return Wst


# token mapping per core: m, g <-> (b, n)
_mm = np.arange(M)
_TOK_B = (_mm // 640) * 128 + (_mm % 128)          # [M]
_TOK_NB = (_mm // 128) % 5                          # [M]
# token-major index: partition p = b % 128, w = (b//128)*50 + nb*10 + g


def _prep_core(static_c, dyn0_c, Wst, consts):
    """Build the per-core in_map. static_c [BC,N,3,T], dyn0_c [BC,N,3]."""
    f32 = np.float32
    xs = np.empty((T, 30, M), f32)
    # row k*10+g, col m = static[b(m), nb(m)*10+g, k, t]
    for g in range(G):
        n_idx = _TOK_NB * 10 + g
        sl = static_c[_TOK_B, n_idx, :, :]          # [M, 3, T]
        for k in range(3):
            xs[:, k * 10 + g, :] = sl[:, k, :].T
    # token-major static planes s0, s2: [T, 128, W]; w = (b//128)*50 + nb*10 + g
    bb = np.arange(BC) // 128
    pp = np.arange(BC) % 128
    s0t = np.empty((T, 128, W), f32)
    s2t = np.empty((T, 128, W), f32)
    wix = (bb[:, None] * 50 + (np.arange(N) // 10 * 10 + np.arange(N) % 10)[None, :])
    # n-index mapping: w = bb*50 + nb*10 + g with n = nb*10+g -> w = bb*50 + n
    for tt in range(T):
        s0t[tt, pp[:, None], bb[:, None] * 50 + np.arange(N)[None, :]] = static_c[:, :, 0, tt]
        s2t[tt, pp[:, None], bb[:, None] * 50 + np.arange(N)[None, :]] = static_c[:, :, 2, tt]
    # dyn0: X rows 30..60 [31, M]; row 30 ones; row 31+ch*10+g
    dyn0 = np.zeros((32, M), f32)
    dyn0[0, :] = 1.0
    for g in range(G):
        n_idx = _TOK_NB * 10 + g
        dl = dyn0_c[_TOK_B, n_idx, :]               # [M, 3]
        for ch in range(3):
            dyn0[2 + ch * 10 + g, :] = dl[:, ch]
    # dmat0 token-major [128, NBLK*30]: blk = bb*5+nb, slot c*10+g
    dmat0 = np.empty((128, NBLK, 3, G), f32)
    n_all = np.arange(N)
    blk = bb[:, None] * 5 + (n_all // 10)[None, :]  # [BC, N]
    gg = (n_all % 10)[None, :].repeat(BC, 0)
    for ch in range(3):
        dmat0[pp[:, None], blk, ch, gg] = dyn0_c[:, :, ch]
    dmat0 = dmat0.reshape(128, NBLK * 30)
    cc = np.zeros((128, 128 + 16 + 9 + NOUT + 1200 + M), f32)
    cc[:, 0:128] = consts["ident"]
    cc[:, 128:144] = consts["cst"]
    cc[:, 144:153] = consts["rvx"]
    cc[0:NROWS, 153:153 + NOUT] = Wst
    cc[:, 253:1453] = dmat0
    cc[0:30, 1453:1453 + M] = xs[0]
    cc[30:62, 1453:1453 + M] = dyn0
    return {"xs": xs, "s0t": s0t, "s2t": s2t, "cc": cc}


def kernel(static, dynamic, Ws, bs, Wd, bd, Wq, bq, Wb, bb, g_s, beta_s,
           m_s, v_s, g_d, beta_d, m_d, v_d):
    static = np.asarray(static, np.float32)
    dyn0 = np.asarray(dynamic, np.float32)[:, :, :, 0]
    Aqs, Aqd, cq, As_z, Ad_z, cz = _fold_weights(
        np.asarray(Ws), np.asarray(bs), np.asarray(Wd), np.asarray(bd),
        np.asarray(Wq), np.asarray(bq), np.asarray(Wb), np.asarray(bb),
        np.asarray(g_s), np.asarray(beta_s), np.asarray(m_s), np.asarray(v_s),
        np.asarray(g_d), np.asarray(beta_d), np.asarray(m_d), np.asarray(v_d))
    Wst = _stationary(Aqs, Aqd, cq, As_z, Ad_z, cz)
    cst = np.zeros((128, 16), np.float32)
    cst[:, 0] = np.nan
    cst[:, 1] = np.inf
    cst[:, 2] = 17.0
    cst[:, 3] = -500.0
    cst[:, 4] = 1.0
    rvx = np.tile((9.0 - np.arange(9, dtype=np.float32))[None, :], (128, 1))
    ident = np.eye(128, dtype=np.float32)
    consts = {"cst": cst, "rvx": rvx, "ident": ident}

    nc = build_core()
    if not nc.is_finalized():
        nc.finalize()
    in_maps = []
    for c in range(NCORES):
        sl = slice(c * BC, (c + 1) * BC)
        in_maps.append(_prep_core(static[sl], dyn0[sl], Wst, consts))
    import os
    trace = bool(os.environ.get("KERNEL_TRACE"))
    res = run_bass_kernel_spmd(nc, in_maps, list(range(NCORES)), trace=trace)
    if trace:
        global LAST_EXEC_NS, LAST_TRACE
        LAST_EXEC_NS = res.exec_time_ns
        LAST_TRACE = getattr(res, "instructions_and_trace", None)
        try:
            print("HW exec time:", res.exec_time_ns, "ns")
        except Exception:
            pass

    # ---- unscramble ----
    action = np.empty((B, N, 3, T), np.float32)
    logp = np.empty((B, N, 3, T), np.float32)
    bb_ = np.arange(BC) // 128
    pp_ = np.arange(BC) % 128
    wix = bb_[:, None] * 50 + np.arange(N)[None, :]   # [BC, N]
    for c in range(NCORES):
        o = res.results[c]["out"].reshape(T, 128, 3, W)
        sl = slice(c * BC, (c + 1) * BC)
        ptr = o[:, pp_[:, None], 0, wix]               # [T, BC, N]
        lq = o[:, pp_[:, None], 1, wix]
        bd_ = o[:, pp_[:, None], 2, wix]
        action[sl, :, 0, :] = 0.0
        action[sl, :, 1, :] = np.moveaxis(ptr, 0, -1)
        action[sl, :, 2, :] = np.moveaxis(bd_, 0, -1)
        logp[sl, :, 0, :] = np.float32(np.log(np.float32(0.5)))
        logp[sl, :, 1, :] = np.moveaxis(lq, 0, -1)
        logp[sl, :, 2, :] = np.moveaxis(bd_, 0, -1)
    return action, logp
